# revision 37
# baseline (speedup 1.0000x reference)
"""Trainium2 Bass kernel for nn_ContrasiveLearningLoss.

Reference computation (f32, N=8192, D=768):
    S = z_a @ z_b.T                       # [N, N]
    diag_i = <z_a[i], z_b[i]>
    loss = sum(S) - 2*sum(diag) + sum(exp(diag))

Algebraic shortcut: sum(S) = <colsum(z_a), colsum(z_b)>, so the whole loss
is O(N*D) and memory-bound.

Sharding: both z_a and z_b are split along the batch dim across 8 cores
(1024 rows each; the two shards are interleaved into one [1024, 1536]
device input per core, row t*128+p holding [z_a row | z_b row]).  Each
core computes partial column sums, partial sum(diag) and partial
sum(exp(diag)); the scalar combine happens on host (the "all-reduce" of
the sharding hint collapses to summing 8 tiny partials).

Per-core device schedule (measured ~34us wall, ~13.4us of which is the
fixed NEFF preamble/tail barrier cost of this runtime):
- 8 sub-tile DMAs [128, 1536] issued round-robin on the two HWDGE rings
  (SP + ACT sequencers) so issue latency overlaps the transfers;
- DVE: one scalar_tensor_tensor per sub-tile = row-wise dot (diag) with
  accumulate, product discarded via stride-0 broadcast write;
- ACT: casts each sub-tile to bf16 (fp32 matmul runs LOW+HIGH dual pass
  on the PE and would be the critical path; bf16 is one pass and PSUM
  still accumulates in fp32);
- PE: column sums as ones[128,1].T @ subtile[128,512], three interleaved
  PSUM accumulation groups; sub-tiles 0-6 accumulate into psA (stored to
  DRAM while the last transfer is in flight), sub-tile 7 into psB so the
  post-last-DMA tail stays short; plus a [128,2] partition-reduce matmul
  for [sum(diag), sum(exp(diag))];
- outputs are DMA'd straight from PSUM (no staging copies).

Numerics note: on this jax/neuron environment the reference produces NaN:
exp(diag) overflows f32 (diag values reach ~117 > log(f32_max)=88.72) and
the XLA reduce over a vector containing +inf yields NaN on the neuron
backend.  The host combine replicates that: if the exp-sum is non-finite
the result is NaN.
"""

import numpy as np
from contextlib import ExitStack

import concourse.bass as bass
import concourse.bacc as bacc
import concourse.tile as tile
from concourse import mybir
from concourse.bass_utils import run_bass_kernel_spmd

N, D = 8192, 768
NCORES = 8
ROWS = N // NCORES          # 1024 rows per core
P = 128                     # SBUF partitions
NT = ROWS // P              # 8 row-tiles of [128, 1536] per core
W = 2 * D                   # combined row width (z_a | z_b)
SEGW = 512                  # matmul segment width (one PSUM bank of f32)
NSEG = W // SEGW            # 3 column classes across the (a|b) width
PBW = W + 2                 # psB row: [colsums | diag_sum | exp_sum]
F32 = mybir.dt.float32
BF16 = mybir.dt.bfloat16

_BUILT = {}


def _build_nc():
    nc = bacc.Bacc("TRN2", target_bir_lowering=False, debug=False)

    # row t*P + p = [z_a row (t*P+p) | z_b row (t*P+p)]
    zab = nc.dram_tensor("zab", [ROWS, W], F32, kind="ExternalInput").ap()
    out_b = nc.dram_tensor("pb", [1, PBW], F32, kind="ExternalOutput").ap()

    zab_t = zab.rearrange("(t p) w -> p t w", p=P)   # [128, NT, W]

    mult = mybir.AluOpType.mult

    with ExitStack() as ctx:
        tc = ctx.enter_context(tile.TileContext(nc))
        data = ctx.enter_context(tc.tile_pool(name="data", bufs=NT))
        bfp = ctx.enter_context(tc.tile_pool(name="bfp", bufs=NT))
        small = ctx.enter_context(tc.tile_pool(name="small", bufs=1))
        psum = ctx.enter_context(tc.tile_pool(name="psum", bufs=1, space="PSUM"))

        ones = small.tile([P, 1], F32)
        nc.vector.memset(ones[:], 1.0)
        ones_bf = small.tile([P, 1], BF16)
        nc.vector.memset(ones_bf[:], 1.0)

        diag_acc = small.tile([P, NT], F32)

        # input loads, alternating between the two HWDGE rings (SP and
        # ACT sequencers) so issue latency overlaps transfers; the last
        # sub-tile is split into two half-width transfers so the
        # post-last-DMA tail chain starts on a half tile
        dtiles = []
        for t in range(NT):
            dt = data.tile([P, W], F32, tag="dt", name=f"dt{t}")
            if t < NT - 1:
                eng = nc.sync if t % 2 == 0 else nc.scalar
                eng.dma_start(dt[:], zab_t[:, t, :])
            else:
                nc.scalar.dma_start(dt[:, 0:D], zab_t[:, t, 0:D])
                nc.sync.dma_start(dt[:, D:W], zab_t[:, t, D:W])
            dtiles.append(dt)

        # column-sum accumulators, bank-aligned so the two staging copies
        # below can run on ACT and DVE in parallel (same-tile PSUM
        # accesses are serialized by the bank tracker)
        psA0 = psum.tile([1, 2 * SEGW], F32, tag="psA0", name="psA0")
        psA1 = psum.tile([1, SEGW], F32, tag="psA1", name="psA1")
        segdst = [psA0[0:1, 0:SEGW], psA0[0:1, SEGW:2 * SEGW], psA1[0:1, 0:SEGW]]
        # psB: [diag_sum, exp_sum] partition reduce
        psB = psum.tile([1, 2], F32, tag="psB", name="psB")

        for t in range(NT):
            dt = dtiles[t]
            # diag partial: row-wise dot product, one DVE pass
            # out = (z_a * 1.0) * z_b elementwise (discarded via stride-0
            # broadcast write), accum_out = row sum
            prod = small.tile([P, 1], F32, tag=f"prod{t}", name=f"prod{t}")
            nc.vector.scalar_tensor_tensor(
                prod[:].broadcast_to((P, D)),
                dt[:, 0:D], 1.0, dt[:, D:W],
                op0=mult, op1=mult,
                accum_out=diag_acc[:, t:t + 1],
            )
            # bf16 cast on ACT (idle during the stream); for the split
            # last sub-tile, cast each half as its DMA lands
            dtbf = bfp.tile([P, W], BF16, tag="dtbf", name=f"dtbf{t}")
            if t < NT - 1:
                nc.scalar.copy(dtbf[:], dt[:])
            else:
                nc.scalar.copy(dtbf[:, 0:D], dt[:, 0:D])
                nc.scalar.copy(dtbf[:, D:W], dt[:, D:W])
            # column sums via PE
            for g in range(NSEG):
                nc.tensor.matmul(segdst[g], ones_bf[:],
                                 dtbf[:, g * SEGW:(g + 1) * SEGW],
                                 start=(t == 0), stop=(t == NT - 1))

        # per-partition sum(diag) and sum(exp(diag)) on ACT, then a
        # partition-reduce matmul
        vecs_sb = small.tile([P, 2], F32)
        scr0 = small.tile([P, NT], F32)
        nc.scalar.activation(scr0[:], diag_acc[:],
                             mybir.ActivationFunctionType.Identity,
                             accum_out=vecs_sb[:, 0:1])
        scr1 = small.tile([P, NT], F32)
        nc.scalar.activation(scr1[:], diag_acc[:],
                             mybir.ActivationFunctionType.Exp,
                             accum_out=vecs_sb[:, 1:2])
        nc.tensor.matmul(psB[:], ones[:], vecs_sb[:], start=True, stop=True)

        # stage PSUM to SBUF with ACT and DVE in parallel, then one
        # output DMA
        sb_out = small.tile([1, PBW], F32)
        nc.scalar.copy(sb_out[0:1, 0:2 * SEGW], psA0[:])
        nc.vector.tensor_copy(sb_out[0:1, 2 * SEGW:W], psA1[:])
        nc.scalar.copy(sb_out[0:1, W:W + 2], psB[:])
        nc.sync.dma_start(out_b[:], sb_out[:])

    nc.compile()
    return nc


def _get_nc():
    if "nc" not in _BUILT:
        _BUILT["nc"] = _build_nc()
    return _BUILT["nc"]


def kernel(z_a: np.ndarray, z_b: np.ndarray) -> np.ndarray:
    z_a = np.ascontiguousarray(np.asarray(z_a, dtype=np.float32))
    z_b = np.ascontiguousarray(np.asarray(z_b, dtype=np.float32))
    assert z_a.shape == (N, D) and z_b.shape == (N, D)

    nc = _get_nc()

    def pack(c):
        sa = z_a[c * ROWS:(c + 1) * ROWS]
        sb = z_b[c * ROWS:(c + 1) * ROWS]
        return np.ascontiguousarray(np.concatenate([sa, sb], axis=1))  # [1024, 1536]

    in_maps = [{"zab": pack(c)} for c in range(NCORES)]
    res = run_bass_kernel_spmd(nc, in_maps, core_ids=list(range(NCORES)))
    outs = res.results

    pb = np.stack([o["pb"][0] for o in outs])             # [8, PBW]

    colsums = pb[:, :W].sum(axis=0, dtype=np.float32)
    colsum_a, colsum_b = colsums[:D], colsums[D:]
    s_sum = np.float32(colsum_a @ colsum_b)
    diag_sum = np.float32(pb[:, W].sum(dtype=np.float32))
    exp_sum = np.float32(pb[:, W + 1].sum(dtype=np.float32))

    total = np.float32(s_sum - np.float32(2.0) * diag_sum + exp_sum)
    if not (np.isfinite(exp_sum) and np.isfinite(total)):
        # exp(diag) overflowed f32.  The reference (jax on neuron) lowers
        # sum() over a vector containing inf to NaN, so match that.
        total = np.float32(np.nan)
    return np.asarray(total, dtype=np.float32)


# revision 38
# speedup vs baseline: 1.0496x; 1.0496x over previous
"""Trainium2 Bass kernel for nn_ContrasiveLearningLoss.

Reference computation (f32, N=8192, D=768):
    S = z_a @ z_b.T                       # [N, N]
    diag_i = <z_a[i], z_b[i]>
    loss = sum(S) - 2*sum(diag) + sum(exp(diag))

Algebraic shortcut: sum(S) = <colsum(z_a), colsum(z_b)>, so the whole loss
is O(N*D) and memory-bound.

Sharding: both z_a and z_b are split along the batch dim across 8 cores
(1024 rows each; the two shards are interleaved into one [1024, 1536]
device input per core, row t*128+p holding [z_a row | z_b row]).  Each
core computes partial column sums, partial sum(diag) and partial
sum(exp(diag)); the scalar combine happens on host (the "all-reduce" of
the sharding hint collapses to summing 8 tiny partials).

Per-core device schedule (measured ~34us wall, ~13.4us of which is the
fixed NEFF preamble/tail barrier cost of this runtime):
- 8 sub-tile DMAs [128, 1536] issued round-robin on the two HWDGE rings
  (SP + ACT sequencers) so issue latency overlaps the transfers;
- DVE: one scalar_tensor_tensor per sub-tile = row-wise dot (diag) with
  accumulate, product discarded via stride-0 broadcast write;
- ACT: casts each sub-tile to bf16 (fp32 matmul runs LOW+HIGH dual pass
  on the PE and would be the critical path; bf16 is one pass and PSUM
  still accumulates in fp32);
- PE: column sums as ones[128,1].T @ subtile[128,512], three interleaved
  PSUM accumulation groups; sub-tiles 0-6 accumulate into psA (stored to
  DRAM while the last transfer is in flight), sub-tile 7 into psB so the
  post-last-DMA tail stays short; plus a [128,2] partition-reduce matmul
  for [sum(diag), sum(exp(diag))];
- outputs are DMA'd straight from PSUM (no staging copies).

Numerics note: on this jax/neuron environment the reference produces NaN:
exp(diag) overflows f32 (diag values reach ~117 > log(f32_max)=88.72) and
the XLA reduce over a vector containing +inf yields NaN on the neuron
backend.  The host combine replicates that: if the exp-sum is non-finite
the result is NaN.
"""

import numpy as np
from contextlib import ExitStack

import concourse.bass as bass
import concourse.bacc as bacc
import concourse.tile as tile
from concourse import mybir
from concourse.bass_utils import run_bass_kernel_spmd

N, D = 8192, 768
NCORES = 8
ROWS = N // NCORES          # 1024 rows per core
P = 128                     # SBUF partitions
NT = ROWS // P              # 8 row-tiles of [128, 1536] per core
W = 2 * D                   # combined row width (z_a | z_b)
SEGW = 512                  # matmul segment width (one PSUM bank of f32)
NSEG = W // SEGW            # 3 column classes across the (a|b) width
PBW = W + 2                 # psB row: [colsums | diag_sum | exp_sum]
F32 = mybir.dt.float32
BF16 = mybir.dt.bfloat16

_BUILT = {}


def _build_nc():
    nc = bacc.Bacc("TRN2", target_bir_lowering=False, debug=False)

    # row t*P + p = [z_a row (t*P+p) | z_b row (t*P+p)]
    zab = nc.dram_tensor("zab", [ROWS, W], F32, kind="ExternalInput").ap()
    out_b = nc.dram_tensor("pb", [1, PBW], F32, kind="ExternalOutput").ap()

    zab_t = zab.rearrange("(t p) w -> p t w", p=P)   # [128, NT, W]

    mult = mybir.AluOpType.mult

    with ExitStack() as ctx:
        tc = ctx.enter_context(tile.TileContext(nc))
        data = ctx.enter_context(tc.tile_pool(name="data", bufs=NT))
        bfp = ctx.enter_context(tc.tile_pool(name="bfp", bufs=NT))
        small = ctx.enter_context(tc.tile_pool(name="small", bufs=1))
        psum = ctx.enter_context(tc.tile_pool(name="psum", bufs=1, space="PSUM"))

        ones = small.tile([P, 1], F32)
        nc.vector.memset(ones[:], 1.0)
        ones_bf = small.tile([P, 1], BF16)
        nc.vector.memset(ones_bf[:], 1.0)

        diag_acc = small.tile([P, NT], F32)

        # input loads, alternating between the two HWDGE rings (SP and
        # ACT sequencers) so issue latency overlaps transfers; the last
        # sub-tile is split into two half-width transfers so the
        # post-last-DMA tail chain starts on a half tile
        dtiles = []
        for t in range(NT):
            dt = data.tile([P, W], F32, tag="dt", name=f"dt{t}")
            if t < NT - 1:
                eng = nc.sync if t % 2 == 0 else nc.scalar
                eng.dma_start(dt[:], zab_t[:, t, :])
            else:
                nc.scalar.dma_start(dt[:, 0:D], zab_t[:, t, 0:D])
                nc.sync.dma_start(dt[:, D:W], zab_t[:, t, D:W])
            dtiles.append(dt)

        # column-sum accumulators, bank-aligned so the two staging copies
        # below can run on ACT and DVE in parallel (same-tile PSUM
        # accesses are serialized by the bank tracker)
        psA0 = psum.tile([1, 2 * SEGW], F32, tag="psA0", name="psA0")
        psA1 = psum.tile([1, SEGW], F32, tag="psA1", name="psA1")
        segdst = [psA0[0:1, 0:SEGW], psA0[0:1, SEGW:2 * SEGW], psA1[0:1, 0:SEGW]]
        # psB: [diag_sum, exp_sum] partition reduce
        psB = psum.tile([1, 2], F32, tag="psB", name="psB")

        for t in range(NT):
            dt = dtiles[t]
            # diag partial: row-wise dot product, one DVE pass
            # out = (z_a * 1.0) * z_b elementwise (discarded via stride-0
            # broadcast write), accum_out = row sum
            prod = small.tile([P, 1], F32, tag=f"prod{t}", name=f"prod{t}")
            nc.vector.scalar_tensor_tensor(
                prod[:].broadcast_to((P, D)),
                dt[:, 0:D], 1.0, dt[:, D:W],
                op0=mult, op1=mult,
                accum_out=diag_acc[:, t:t + 1],
            )
            # bf16 cast on ACT (idle during the stream); for the split
            # last sub-tile, cast each half as its DMA lands
            dtbf = bfp.tile([P, W], BF16, tag="dtbf", name=f"dtbf{t}")
            if t < NT - 1:
                nc.scalar.copy(dtbf[:], dt[:])
            else:
                nc.scalar.copy(dtbf[:, 0:D], dt[:, 0:D])
                nc.scalar.copy(dtbf[:, D:W], dt[:, D:W])
            # column sums via PE
            for g in range(NSEG):
                nc.tensor.matmul(segdst[g], ones_bf[:],
                                 dtbf[:, g * SEGW:(g + 1) * SEGW],
                                 start=(t == 0), stop=(t == NT - 1))

        # per-partition sum(diag) and sum(exp(diag)) on ACT, then a
        # partition-reduce matmul
        vecs_sb = small.tile([P, 2], F32)
        scr0 = small.tile([P, NT], F32)
        nc.scalar.activation(scr0[:], diag_acc[:],
                             mybir.ActivationFunctionType.Identity,
                             accum_out=vecs_sb[:, 0:1])
        scr1 = small.tile([P, NT], F32)
        nc.scalar.activation(scr1[:], diag_acc[:],
                             mybir.ActivationFunctionType.Exp,
                             accum_out=vecs_sb[:, 1:2])
        nc.tensor.matmul(psB[:], ones[:], vecs_sb[:], start=True, stop=True)

        # stage PSUM to SBUF with ACT and DVE in parallel, then one
        # output DMA
        sb_out = small.tile([1, PBW], F32)
        nc.scalar.copy(sb_out[0:1, 0:2 * SEGW], psA0[:])
        nc.vector.tensor_copy(sb_out[0:1, 2 * SEGW:W], psA1[:])
        nc.scalar.copy(sb_out[0:1, W:W + 2], psB[:])
        nc.sync.dma_start(out_b[:], sb_out[:])

    nc.compile()
    return nc


def _get_nc():
    if "nc" not in _BUILT:
        _BUILT["nc"] = _build_nc()
    return _BUILT["nc"]


def kernel(z_a: np.ndarray, z_b: np.ndarray) -> np.ndarray:
    z_a = np.ascontiguousarray(np.asarray(z_a, dtype=np.float32))
    z_b = np.ascontiguousarray(np.asarray(z_b, dtype=np.float32))
    assert z_a.shape == (N, D) and z_b.shape == (N, D)

    nc = _get_nc()

    def pack(c):
        sa = z_a[c * ROWS:(c + 1) * ROWS]
        sb = z_b[c * ROWS:(c + 1) * ROWS]
        return np.ascontiguousarray(np.concatenate([sa, sb], axis=1))  # [1024, 1536]

    in_maps = [{"zab": pack(c)} for c in range(NCORES)]
    try:
        res = run_bass_kernel_spmd(nc, in_maps, core_ids=list(range(NCORES)))
    except Exception:
        # transient PJRT/terminal hiccups have been observed in this
        # environment; one retry with a freshly built program
        _BUILT.clear()
        nc = _get_nc()
        res = run_bass_kernel_spmd(nc, in_maps, core_ids=list(range(NCORES)))
    outs = res.results

    pb = np.stack([o["pb"][0] for o in outs])             # [8, PBW]

    colsums = pb[:, :W].sum(axis=0, dtype=np.float32)
    colsum_a, colsum_b = colsums[:D], colsums[D:]
    s_sum = np.float32(colsum_a @ colsum_b)
    diag_sum = np.float32(pb[:, W].sum(dtype=np.float32))
    exp_sum = np.float32(pb[:, W + 1].sum(dtype=np.float32))

    total = np.float32(s_sum - np.float32(2.0) * diag_sum + exp_sum)
    if not (np.isfinite(exp_sum) and np.isfinite(total)):
        # exp(diag) overflowed f32.  The reference (jax on neuron) lowers
        # sum() over a vector containing inf to NaN, so match that.
        total = np.float32(np.nan)
    return np.asarray(total, dtype=np.float32)
